# revision 13
# baseline (speedup 1.0000x reference)
"""Self-contained Trainium2 Bass kernel for nn_AttentionBlock (GroupNorm +
single-head attention + residual).

Reference computation (shapes hardcoded):
    x: [B=4, H=64, W=64, C=256] f32
    xn = GroupNorm(x, groups=8, eps=1e-3) * gamma + beta
    q/k/v = xn @ W{q,k,v} + b{q,k,v}           # per batch, N=H*W=4096 tokens
    attn = softmax(q @ k^T / sqrt(C))
    out  = xn + (attn @ v) @ Wp + bp

Sharding: 8 cores = (batch b, query-half h). Each core receives its batch's
full token sequence with rows rotated so its 2048 query rows come first,
computes groupnorm + K/V for all 4096 tokens and attention for its 2048
query rows. No collectives. Host reassembles the 8 [2048, 256] outputs.
"""

import numpy as np

import concourse.bass as bass
import concourse.tile as tile
from concourse import mybir
from concourse.bass_utils import run_bass_kernel_spmd
from concourse.tile import ScopedClock

# Problem shapes (hardcoded per contract)
B, H, W, C = 4, 64, 64, 256
N = H * W            # 4096 tokens per batch image
NQ = N // 2          # 2048 query rows per core
G = 8                # groupnorm groups
CG = C // G          # 32 channels per group
EPS = 1e-3
P = 128
FD = 512             # matmul moving-operand free dim (one PSUM bank of f32)
NKB = N // P         # 32 key blocks
NQT = NQ // FD       # 4 query tiles per core
SCALE = float(C) ** -0.5
F32 = mybir.dt.float32
AF = mybir.ActivationFunctionType


def _drain_and_barrier_split(self, tick_clock, wait_clock):
    """Replacement for TileContext._drain_and_barrier.

    The walrus build in this container rejects sem waits on InstDrain (and
    >1 wait on a NOP), so carry the end-of-kernel waits on a chain of NOPs
    with one wait each, drain without sync, and use the sem-only (no-Drain)
    all-engine barrier around semaphore cleanup.
    """
    nc = self.nc
    carrier = nc.sync.nop(nofuse=True)
    wait_clock.add_sem_waits(
        carrier.ins, ScopedClock({None: tick_clock.global_clock})
    )
    si = carrier.ins.sync_info
    waits = list(si.on_wait) if si is not None and si.on_wait else []
    if len(waits) > 1:
        carrier.ins.sync_info = mybir.SyncInfo(
            on_wait=waits[:1], on_update=list(si.on_update or [])
        )
        for w in waits[1:]:
            extra = nc.sync.nop(nofuse=True)
            extra.ins.sync_info = mybir.SyncInfo(on_wait=[w], on_update=[])
    nc.sync.drain()
    nc.all_engine_barrier(sem_only=True)
    assert self.sems is not None
    popped = nc._tile_sem_poison_stack.pop()
    assert popped is self._sem_poison
    nc.clear_and_free_semaphores(list(self.sems.allocated().values()))
    nc.all_engine_barrier(sem_only=True)


tile.TileContext._drain_and_barrier = _drain_and_barrier_split

_wsplit_ctr = 0


def _split_multi_waits(nc: bass.Bass):
    """Walrus in this container supports at most one sync wait per
    instruction (and none on Drain). Hoist excess waits onto NoOps placed
    just before the instruction on the same engine — sequencers process
    instructions in order, so blocking on the NoOp is equivalent."""
    global _wsplit_ctr
    for f in nc.m.functions:
        for bb in f.blocks:
            new_insts = []
            for ins in bb.instructions:
                si = getattr(ins, "sync_info", None)
                waits = list(si.on_wait) if si is not None and si.on_wait else []
                limit = 0 if ins.opcode == "Drain" else 1
                if len(waits) > limit:
                    keep = waits[len(waits) - limit:] if limit else []
                    hoist = waits[: len(waits) - limit]
                    for w in hoist:
                        _wsplit_ctr += 1
                        nop = mybir.InstNoOp(
                            name=f"I-wsplit-{_wsplit_ctr}",
                            engine=ins.engine,
                            sync_info=mybir.SyncInfo(on_wait=[w], on_update=[]),
                        )
                        new_insts.append(nop)
                    ins.sync_info = mybir.SyncInfo(
                        on_wait=keep, on_update=list(si.on_update or [])
                    )
                new_insts.append(ins)
            bb.instructions[:] = new_insts


def build_nc(split_waits: bool = True) -> bass.Bass:
    nc = bass.Bass()
    x = nc.declare_dram_parameter("x", [N, C], F32, isOutput=False)[:]
    wq = nc.declare_dram_parameter("wq", [C, C], F32, isOutput=False)[:]
    wk = nc.declare_dram_parameter("wk", [C, C], F32, isOutput=False)[:]
    wv = nc.declare_dram_parameter("wv", [C, C], F32, isOutput=False)[:]
    wp = nc.declare_dram_parameter("wp", [C, C], F32, isOutput=False)[:]
    bq = nc.declare_dram_parameter("bq", [C], F32, isOutput=False)[:]
    bk = nc.declare_dram_parameter("bk", [C], F32, isOutput=False)[:]
    bv = nc.declare_dram_parameter("bv", [C], F32, isOutput=False)[:]
    bp = nc.declare_dram_parameter("bp", [C], F32, isOutput=False)[:]
    gamma = nc.declare_dram_parameter("gamma", [C], F32, isOutput=False)[:]
    beta = nc.declare_dram_parameter("beta", [C], F32, isOutput=False)[:]
    ident = nc.declare_dram_parameter("ident", [P, P], F32, isOutput=False)[:]
    egrp = nc.declare_dram_parameter("egrp", [P, 2 * G], F32, isOutput=False)[:]
    egrpt = nc.declare_dram_parameter("egrpt", [G, C], F32, isOutput=False)[:]
    out = nc.declare_dram_parameter("out", [NQ, C], F32, isOutput=True)[:]

    with tile.TileContext(nc) as tc:
        _body(nc, tc, x, wq, wk, wv, wp, bq, bk, bv, bp, gamma, beta, ident,
              egrp, egrpt, out)
    if split_waits:
        _split_multi_waits(nc)
    return nc


def _body(nc, tc, x, wq, wk, wv, wp, bq, bk, bv, bp, gamma, beta, ident,
          egrp, egrpt, out):
    from contextlib import ExitStack
    ctx = ExitStack()
    with ctx:
        consts = ctx.enter_context(tc.tile_pool(name="consts", bufs=1))
        big = ctx.enter_context(tc.tile_pool(name="big", bufs=1))
        work = ctx.enter_context(tc.tile_pool(name="work", bufs=3))
        small = ctx.enter_context(tc.tile_pool(name="small", bufs=2))

        # ---- constants ----
        ident_sb = consts.tile([P, P], F32, tag="ident")
        nc.sync.dma_start(out=ident_sb, in_=ident)
        ones_sb = consts.tile([P, 1], F32, tag="ones")
        nc.vector.memset(ones_sb, 1.0)
        ones_row = consts.tile([1, P], F32, tag="ones_row")
        nc.vector.memset(ones_row, 1.0)

        w_sbs = {}
        for name, wh in (("q", wq), ("k", wk), ("v", wv), ("p", wp)):
            t = consts.tile([P, 2, C], F32, tag=f"w{name}")
            nc.sync.dma_start(out=t, in_=wh.rearrange("(kb p) co -> p kb co", p=P))
            w_sbs[name] = t
        bq_pp = consts.tile([P, 2], F32, tag="bq")
        nc.sync.dma_start(out=bq_pp, in_=bq.rearrange("(b p) -> p b", p=P))
        bk_pp = consts.tile([P, 2], F32, tag="bk")
        nc.sync.dma_start(out=bk_pp, in_=bk.rearrange("(b p) -> p b", p=P))
        bp_pp = consts.tile([P, 2], F32, tag="bp")
        nc.sync.dma_start(out=bp_pp, in_=bp.rearrange("(b p) -> p b", p=P))
        bv_bc = consts.tile([P, C], F32, tag="bv")
        nc.gpsimd.dma_start(
            out=bv_bc,
            in_=bass.AP(tensor=bv.tensor, offset=bv.offset, ap=[[0, P], [1, C]]),
        )
        gamma_pp = consts.tile([P, 2], F32, tag="gamma")
        nc.sync.dma_start(out=gamma_pp, in_=gamma.rearrange("(b p) -> p b", p=P))
        beta_pp = consts.tile([P, 2], F32, tag="beta")
        nc.sync.dma_start(out=beta_pp, in_=beta.rearrange("(b p) -> p b", p=P))
        egrp_sb = consts.tile([P, 2 * G], F32, tag="egrp")
        nc.sync.dma_start(out=egrp_sb, in_=egrp)
        egrpt_sb = consts.tile([G, C], F32, tag="egrpt")
        nc.sync.dma_start(out=egrpt_sb, in_=egrpt)

        # ---- phase A: load x, stats, transpose ----
        xnT = big.tile([P, 2, N], F32, tag="xnT")

        with tc.tile_pool(name="xnat", bufs=1) as xnat_pool, \
             tc.tile_pool(name="psA", bufs=1, space="PSUM") as psA, \
             tc.tile_pool(name="psT", bufs=2, space="PSUM") as psT:
            x_nat = xnat_pool.tile([P, N // P, C], F32, tag="xnat")
            nc.sync.dma_start(out=x_nat, in_=x.rearrange("(t p) c -> p t c", p=P))

            # raw transpose x -> xT (normalize applied in-place later)
            for t in range(N // P):
                for cb in range(2):
                    ps_t = psT.tile([P, P], F32, tag="tp")
                    nc.tensor.transpose(ps_t, x_nat[:, t, cb * P:(cb + 1) * P],
                                        ident_sb)
                    nc.vector.tensor_copy(out=xnT[:, cb, t * P:(t + 1) * P],
                                          in_=ps_t)

            # per-channel mean/var via bn_stats on x^T (channels on partitions)
            fmax = nc.vector.BN_STATS_FMAX
            nsub = N // fmax
            ps_g = psA.tile([G, 2], F32, tag="gstats")
            for cb in range(2):
                stats = work.tile([P, nsub, nc.vector.BN_STATS_DIM], F32,
                                  tag="bnstats")
                for s in range(nsub):
                    nc.vector.bn_stats(
                        out=stats[:, s, :],
                        in_=xnT[:, cb, s * fmax:(s + 1) * fmax])
                mv = work.tile([P, nc.vector.BN_AGGR_DIM], F32, tag="bnmv")
                nc.vector.bn_aggr(out=mv, in_=stats)
                # pack (mean_c, E[x^2]_c = var_c + mean_c^2)
                pk = work.tile([P, 2], F32, tag="pk")
                nc.vector.tensor_copy(out=pk[:, 0:1], in_=mv[:, 0:1])
                msq = work.tile([P, 1], F32, tag="msq")
                nc.vector.tensor_mul(out=msq, in0=mv[:, 0:1], in1=mv[:, 0:1])
                nc.vector.tensor_add(out=pk[:, 1:2], in0=mv[:, 1:2], in1=msq)
                # group-sum across channel partitions (one-hot matmul)
                nc.tensor.matmul(ps_g, lhsT=egrp_sb[:, cb * G:(cb + 1) * G],
                                 rhs=pk, start=(cb == 0), stop=(cb == 1),
                                 skip_group_check=True)

            # finalize on G partitions: mean_g, rstd_g
            gsb = small.tile([G, 2], F32, tag="gsb")
            nc.vector.tensor_scalar_mul(gsb, ps_g, 1.0 / CG)
            gmean = gsb[:, 0:1]
            gex2 = gsb[:, 1:2]
            gmsq = small.tile([G, 1], F32, tag="gmsq")
            nc.vector.tensor_mul(out=gmsq, in0=gmean, in1=gmean)
            gvar = small.tile([G, 1], F32, tag="gvar")
            nc.vector.tensor_tensor(out=gvar, in0=gex2, in1=gmsq,
                                    op=mybir.AluOpType.subtract)
            eps_sb = small.tile([G, 1], F32, tag="eps")
            nc.vector.memset(eps_sb, EPS)
            gstd = small.tile([G, 1], F32, tag="gstd")
            nc.scalar.activation(out=gstd, in_=gvar, func=AF.Sqrt, bias=eps_sb)
            gpack = small.tile([G, 2], F32, tag="gpack")
            nc.vector.tensor_copy(out=gpack[:, 0:1], in_=gmean)
            nc.vector.reciprocal(out=gpack[:, 1:2], in_=gstd)

            # broadcast group stats back to channel partitions
            all4 = consts.tile([P, 4], F32, tag="all4")
            for cb in range(2):
                ps_bc = psA.tile([P, 2], F32, tag="bc")
                nc.tensor.matmul(ps_bc, lhsT=egrpt_sb[:, cb * P:(cb + 1) * P],
                                 rhs=gpack, start=True, stop=True)
                mr = small.tile([P, 2], F32, tag="mr")
                nc.vector.tensor_copy(out=mr, in_=ps_bc)
                # scale_c = rstd_g(c) * gamma_c ; shift_c = beta_c - mean*scale
                nc.vector.tensor_mul(out=all4[:, cb:cb + 1], in0=mr[:, 1:2],
                                     in1=gamma_pp[:, cb:cb + 1])
                ms = small.tile([P, 1], F32, tag="ms")
                nc.vector.tensor_mul(out=ms, in0=mr[:, 0:1],
                                     in1=all4[:, cb:cb + 1])
                nc.vector.tensor_tensor(out=all4[:, 2 + cb:3 + cb],
                                        in0=beta_pp[:, cb:cb + 1], in1=ms,
                                        op=mybir.AluOpType.subtract)

            # normalize xT in place: xn^T = x^T * scale_c + shift_c
            for cb in range(2):
                nc.vector.tensor_scalar(
                    out=xnT[:, cb, :], in0=xnT[:, cb, :],
                    scalar1=all4[:, cb:cb + 1], scalar2=all4[:, 2 + cb:3 + cb],
                    op0=mybir.AluOpType.mult, op1=mybir.AluOpType.add)

        # ---- phase B: q^T, k^T (weights stationary), v natural ----
        qT = big.tile([P, 2, NQ], F32, tag="qT")
        kT = big.tile([P, 2, N], F32, tag="kT")
        v_sb = big.tile([P, N // P, C], F32, tag="v")

        with tc.tile_pool(name="psB", bufs=2, space="PSUM") as psB:
            for cob in range(2):
                for nt in range(NQ // FD):
                    ps = psB.tile([P, FD], F32, tag="qk")
                    for kb in range(2):
                        nc.tensor.matmul(
                            ps, lhsT=w_sbs["q"][:, kb, cob * P:(cob + 1) * P],
                            rhs=xnT[:, kb, nt * FD:(nt + 1) * FD],
                            start=(kb == 0), stop=(kb == 1))
                    nc.vector.tensor_scalar_add(
                        qT[:, cob, nt * FD:(nt + 1) * FD], ps,
                        bq_pp[:, cob:cob + 1])
                for nt in range(N // FD):
                    ps = psB.tile([P, FD], F32, tag="qk")
                    for kb in range(2):
                        nc.tensor.matmul(
                            ps, lhsT=w_sbs["k"][:, kb, cob * P:(cob + 1) * P],
                            rhs=xnT[:, kb, nt * FD:(nt + 1) * FD],
                            start=(kb == 0), stop=(kb == 1))
                    nc.vector.tensor_scalar_add(
                        kT[:, cob, nt * FD:(nt + 1) * FD], ps,
                        bk_pp[:, cob:cob + 1])
            for rb in range(N // P):
                psv = psB.tile([P, C], F32, tag="vps")
                for kb in range(2):
                    nc.tensor.matmul(psv, lhsT=xnT[:, kb, rb * P:(rb + 1) * P],
                                     rhs=w_sbs["v"][:, kb, :],
                                     start=(kb == 0), stop=(kb == 1))
                nc.vector.tensor_add(out=v_sb[:, rb, :], in0=psv, in1=bv_bc)

        # ---- phase C: attention per query tile of FD rows ----
        with tc.tile_pool(name="psS", bufs=2, space="PSUM") as psS, \
             tc.tile_pool(name="psO", bufs=1, space="PSUM") as psO, \
             tc.tile_pool(name="psC", bufs=1, space="PSUM") as psC, \
             tc.tile_pool(name="psM", bufs=1, space="PSUM") as psM, \
             tc.tile_pool(name="epool", bufs=4) as epool, \
             tc.tile_pool(name="apool", bufs=2) as apool, \
             tc.tile_pool(name="fpool", bufs=2) as fpool, \
             tc.tile_pool(name="opool", bufs=3) as opool:
            for qt in range(NQT):
                qs = qt * FD
                ps_o0 = psO.tile([P, FD], F32, tag="o0")
                ps_o1 = psO.tile([P, FD], F32, tag="o1")
                ps_c = psC.tile([1, FD], F32, tag="c")
                for kb in range(NKB):
                    ps_s = psS.tile([P, FD], F32, tag="s")
                    nc.tensor.matmul(ps_s, lhsT=kT[:, 0, kb * P:(kb + 1) * P],
                                     rhs=qT[:, 0, qs:qs + FD],
                                     start=True, stop=False)
                    nc.tensor.matmul(ps_s, lhsT=kT[:, 1, kb * P:(kb + 1) * P],
                                     rhs=qT[:, 1, qs:qs + FD],
                                     start=False, stop=True)
                    eT = epool.tile([P, FD], F32, tag="eT")
                    nc.scalar.activation(out=eT, in_=ps_s, func=AF.Exp,
                                         scale=SCALE)
                    last = kb == NKB - 1
                    nc.tensor.matmul(ps_o0, lhsT=v_sb[:, kb, 0:P], rhs=eT,
                                     start=(kb == 0), stop=last,
                                     skip_group_check=True)
                    nc.tensor.matmul(ps_o1, lhsT=v_sb[:, kb, P:C], rhs=eT,
                                     start=(kb == 0), stop=last,
                                     skip_group_check=True)
                    nc.tensor.matmul(ps_c, lhsT=ones_sb, rhs=eT,
                                     start=(kb == 0), stop=last,
                                     skip_group_check=True)
                # normalize: attn^T = outT_unnorm * (1/colsum) broadcast
                rec = small.tile([1, FD], F32, tag="rec")
                nc.vector.reciprocal(out=rec, in_=ps_c)
                ps_r = psM.tile([P, FD], F32, tag="r")
                nc.tensor.matmul(ps_r, lhsT=ones_row, rhs=rec,
                                 start=True, stop=True)
                rec_bc = work.tile([P, FD], F32, tag="rec_bc")
                nc.vector.tensor_copy(out=rec_bc, in_=ps_r)
                aT = apool.tile([P, 2, FD], F32, tag="aT")
                nc.vector.tensor_mul(out=aT[:, 0, :], in0=ps_o0, in1=rec_bc)
                nc.vector.tensor_mul(out=aT[:, 1, :], in0=ps_o1, in1=rec_bc)
                # proj (Wp stationary) + bp + residual xn^T
                fos = []
                for cob in range(2):
                    ps_f = psM.tile([P, FD], F32, tag="f")
                    for kb2 in range(2):
                        nc.tensor.matmul(
                            ps_f, lhsT=w_sbs["p"][:, kb2, cob * P:(cob + 1) * P],
                            rhs=aT[:, kb2, :], start=(kb2 == 0), stop=(kb2 == 1))
                    fo = fpool.tile([P, FD], F32, tag=f"fo{cob}")
                    nc.vector.tensor_scalar_add(fo, ps_f, bp_pp[:, cob:cob + 1])
                    nc.vector.tensor_add(out=fo, in0=fo,
                                         in1=xnT[:, cob, qs:qs + FD])
                    fos.append(fo)
                # transpose back to natural [rows, C] and store
                for qb in range(FD // P):
                    onat = opool.tile([P, C], F32, tag="onat")
                    for cob in range(2):
                        ps_ot = psM.tile([P, P], F32, tag="ot")
                        nc.tensor.transpose(ps_ot,
                                            fos[cob][:, qb * P:(qb + 1) * P],
                                            ident_sb)
                        nc.vector.tensor_copy(out=onat[:, cob * P:(cob + 1) * P],
                                              in_=ps_ot)
                    r0 = qs + qb * P
                    nc.sync.dma_start(out=out[r0:r0 + P, :], in_=onat)


_NC_CACHE = None


def _get_nc():
    global _NC_CACHE
    if _NC_CACHE is None:
        _NC_CACHE = build_nc()
    return _NC_CACHE


def _egrp_const() -> np.ndarray:
    """[P, 2G] one-hot: egrp[p, cb*G+g] = 1 iff channel cb*P+p is in group g."""
    e = np.zeros((P, 2 * G), dtype=np.float32)
    for cb in range(2):
        for p in range(P):
            e[p, cb * G + (cb * P + p) // CG] = 1.0
    return e


def _egrpt_const() -> np.ndarray:
    """[G, C] one-hot transpose: egrpt[g, c] = 1 iff group(c) == g."""
    e = np.zeros((G, C), dtype=np.float32)
    for c in range(C):
        e[c // CG, c] = 1.0
    return e


def make_in_maps(inputs: dict) -> list[dict]:
    x = np.ascontiguousarray(np.asarray(inputs["x"], dtype=np.float32))
    x_flat = x.reshape(B, N, C)
    shared = {
        "wq": np.asarray(inputs["Wq"], np.float32),
        "wk": np.asarray(inputs["Wk"], np.float32),
        "wv": np.asarray(inputs["Wv"], np.float32),
        "wp": np.asarray(inputs["Wp"], np.float32),
        "bq": np.asarray(inputs["bq"], np.float32),
        "bk": np.asarray(inputs["bk"], np.float32),
        "bv": np.asarray(inputs["bv"], np.float32),
        "bp": np.asarray(inputs["bp"], np.float32),
        "gamma": np.asarray(inputs["gamma"], np.float32),
        "beta": np.asarray(inputs["beta"], np.float32),
        "ident": np.eye(P, dtype=np.float32),
        "egrp": _egrp_const(),
        "egrpt": _egrpt_const(),
    }
    in_maps = []
    for core in range(8):
        b, h = core // 2, core % 2
        if h == 0:
            xp = x_flat[b]
        else:
            xp = np.concatenate([x_flat[b, NQ:], x_flat[b, :NQ]], axis=0)
        in_maps.append({"x": np.ascontiguousarray(xp), **shared})
    return in_maps


def assemble(results: list[dict]) -> np.ndarray:
    y = np.empty((B, N, C), dtype=np.float32)
    for core in range(8):
        b, h = core // 2, core % 2
        y[b, h * NQ:(h + 1) * NQ] = results[core]["out"]
    return y.reshape(B, H, W, C)


def kernel(**inputs) -> np.ndarray:
    nc = _get_nc()
    res = run_bass_kernel_spmd(nc, make_in_maps(inputs), core_ids=list(range(8)))
    return assemble(res.results)


# revision 25
# speedup vs baseline: 4.2720x; 4.2720x over previous
"""Self-contained Trainium2 Bass kernel for nn_AttentionBlock (GroupNorm +
single-head attention + residual).

Reference computation (shapes hardcoded):
    x: [B=4, H=64, W=64, C=256] f32
    xn = GroupNorm(x, groups=8, eps=1e-3) * gamma + beta
    q/k/v = xn @ W{q,k,v} + b{q,k,v}           # per batch, N=H*W=4096 tokens
    attn = softmax(q @ k^T / sqrt(C))
    out  = xn + (attn @ v) @ Wp + bp

Sharding: 8 cores = (batch b, query-half h). Each core receives its batch's
full token sequence with rows rotated so its 2048 query rows come first,
computes groupnorm + K/V for all 4096 tokens and attention for its 2048
query rows. No collectives. Host reassembles the 8 [2048, 256] outputs.
"""

import numpy as np

import concourse.bass as bass
import concourse.tile as tile
from concourse import mybir
from concourse.bass_utils import run_bass_kernel_spmd
from concourse.tile import ScopedClock

# Problem shapes (hardcoded per contract)
B, H, W, C = 4, 64, 64, 256
N = H * W            # 4096 tokens per batch image
NQ = N // 2          # 2048 query rows per core
G = 8                # groupnorm groups
CG = C // G          # 32 channels per group
EPS = 1e-3
P = 128
FD = 512             # matmul moving-operand free dim (one PSUM bank of f32)
NKB = N // P         # 32 key blocks
NQT = NQ // FD       # 4 query tiles per core
SCALE = float(C) ** -0.5
F32 = mybir.dt.float32
AF = mybir.ActivationFunctionType

# dev knob for phase bisection:
# "load" (DMAs only) | "ldtp" (+transposes) | "ldst" (+stats/normalize)
# | "ab" (+qkv) | "abs" (+scores/exp) | "ab1" (1 q-tile) | "full"
VARIANT = "full"
_STAGES = {"load": 0, "ldtp": 1, "ldst": 2, "ab": 3, "abs": 4, "ab1": 5,
           "full": 5}


def _drain_and_barrier_split(self, tick_clock, wait_clock):
    """Replacement for TileContext._drain_and_barrier.

    The walrus build in this container rejects sem waits on InstDrain (and
    >1 wait on a NOP), so carry the end-of-kernel waits on a chain of NOPs
    with one wait each, drain without sync, and use the sem-only (no-Drain)
    all-engine barrier around semaphore cleanup.
    """
    nc = self.nc
    carrier = nc.sync.nop(nofuse=True)
    wait_clock.add_sem_waits(
        carrier.ins, ScopedClock({None: tick_clock.global_clock})
    )
    si = carrier.ins.sync_info
    waits = list(si.on_wait) if si is not None and si.on_wait else []
    if len(waits) > 1:
        carrier.ins.sync_info = mybir.SyncInfo(
            on_wait=waits[:1], on_update=list(si.on_update or [])
        )
        for w in waits[1:]:
            extra = nc.sync.nop(nofuse=True)
            extra.ins.sync_info = mybir.SyncInfo(on_wait=[w], on_update=[])
    nc.sync.drain()
    nc.all_engine_barrier(sem_only=True)
    assert self.sems is not None
    popped = nc._tile_sem_poison_stack.pop()
    assert popped is self._sem_poison
    nc.clear_and_free_semaphores(list(self.sems.allocated().values()))
    nc.all_engine_barrier(sem_only=True)


tile.TileContext._drain_and_barrier = _drain_and_barrier_split

_wsplit_ctr = 0


def _split_multi_waits(nc: bass.Bass):
    """Walrus in this container supports at most one sync wait per
    instruction (and none on Drain). Hoist excess waits onto NoOps placed
    just before the instruction on the same engine — sequencers process
    instructions in order, so blocking on the NoOp is equivalent."""
    global _wsplit_ctr
    for f in nc.m.functions:
        for bb in f.blocks:
            new_insts = []
            for ins in bb.instructions:
                si = getattr(ins, "sync_info", None)
                waits = list(si.on_wait) if si is not None and si.on_wait else []
                limit = 0 if ins.opcode == "Drain" else 1
                if len(waits) > limit:
                    keep = waits[len(waits) - limit:] if limit else []
                    hoist = waits[: len(waits) - limit]
                    for w in hoist:
                        _wsplit_ctr += 1
                        nop = mybir.InstNoOp(
                            name=f"I-wsplit-{_wsplit_ctr}",
                            engine=ins.engine,
                            sync_info=mybir.SyncInfo(on_wait=[w], on_update=[]),
                        )
                        new_insts.append(nop)
                    ins.sync_info = mybir.SyncInfo(
                        on_wait=keep, on_update=list(si.on_update or [])
                    )
                new_insts.append(ins)
            bb.instructions[:] = new_insts


# ---- single-blob input packing (one input param + one output param:
# each extra parameter costs ~2 ms/execution in this PJRT path) ----
_SEGS = [
    ("x", N * C),
    ("wq", C * C), ("wk", C * C), ("wv", C * C), ("wp", C * C),
    ("bq", C), ("bk", C), ("bv", C), ("bp", C),
    ("gamma", C), ("beta", C),
    ("ident", P * P),
    ("egrp", P * 2 * G),
    ("egrpt", G * C),
]
_OFF = {}
_total = 0
for _nm, _sz in _SEGS:
    _OFF[_nm] = _total
    _total += _sz
BLOB_SIZE = _total


def build_nc(split_waits: bool = True) -> bass.Bass:
    nc = bass.Bass()
    blob = nc.declare_dram_parameter("blob", [BLOB_SIZE], F32, isOutput=False)[:]

    def seg(name, size):
        return blob[_OFF[name]:_OFF[name] + size]

    x = seg("x", N * C).rearrange("(p t c) -> p t c", p=P, c=C)
    wq = seg("wq", C * C).rearrange("(ci co) -> ci co", co=C)
    wk = seg("wk", C * C).rearrange("(ci co) -> ci co", co=C)
    wv = seg("wv", C * C).rearrange("(ci co) -> ci co", co=C)
    wp = seg("wp", C * C).rearrange("(ci co) -> ci co", co=C)
    bq = seg("bq", C)
    bk = seg("bk", C)
    bv = seg("bv", C)
    bp = seg("bp", C)
    gamma = seg("gamma", C)
    beta = seg("beta", C)
    ident = seg("ident", P * P).rearrange("(a b) -> a b", b=P)
    egrp = seg("egrp", P * 2 * G).rearrange("(a b) -> a b", b=2 * G)
    egrpt = seg("egrpt", G * C).rearrange("(a b) -> a b", b=C)
    out = nc.declare_dram_parameter("out", [NQ, C], F32, isOutput=True)[:]

    with tile.TileContext(nc) as tc:
        _body(nc, tc, x, wq, wk, wv, wp, bq, bk, bv, bp, gamma, beta, ident,
              egrp, egrpt, out)
    if split_waits:
        _split_multi_waits(nc)
    return nc


def _body(nc, tc, x, wq, wk, wv, wp, bq, bk, bv, bp, gamma, beta, ident,
          egrp, egrpt, out):
    from contextlib import ExitStack
    ctx = ExitStack()
    with ctx:
        consts = ctx.enter_context(tc.tile_pool(name="consts", bufs=1))
        big = ctx.enter_context(tc.tile_pool(name="big", bufs=1))
        work = ctx.enter_context(tc.tile_pool(name="work", bufs=3))
        small = ctx.enter_context(tc.tile_pool(name="small", bufs=2))

        # ---- constants ----
        ident_sb = consts.tile([P, P], F32, tag="ident")
        nc.sync.dma_start(out=ident_sb, in_=ident)
        ones_sb = consts.tile([P, 1], F32, tag="ones")
        nc.vector.memset(ones_sb, 1.0)
        ones_row = consts.tile([1, P], F32, tag="ones_row")
        nc.vector.memset(ones_row, 1.0)

        w_sbs = {}
        for name, wh in (("q", wq), ("k", wk), ("v", wv), ("p", wp)):
            t = consts.tile([P, 2, C], F32, tag=f"w{name}")
            nc.sync.dma_start(out=t, in_=wh.rearrange("(kb p) co -> p kb co", p=P))
            w_sbs[name] = t
        bq_pp = consts.tile([P, 2], F32, tag="bq")
        nc.sync.dma_start(out=bq_pp, in_=bq.rearrange("(b p) -> p b", p=P))
        bk_pp = consts.tile([P, 2], F32, tag="bk")
        nc.sync.dma_start(out=bk_pp, in_=bk.rearrange("(b p) -> p b", p=P))
        bp_pp = consts.tile([P, 2], F32, tag="bp")
        nc.sync.dma_start(out=bp_pp, in_=bp.rearrange("(b p) -> p b", p=P))
        bv_bc = consts.tile([P, C], F32, tag="bv")
        nc.gpsimd.dma_start(
            out=bv_bc,
            in_=bass.AP(tensor=bv.tensor, offset=bv.offset, ap=[[0, P], [1, C]]),
        )
        gamma_pp = consts.tile([P, 2], F32, tag="gamma")
        nc.sync.dma_start(out=gamma_pp, in_=gamma.rearrange("(b p) -> p b", p=P))
        beta_pp = consts.tile([P, 2], F32, tag="beta")
        nc.sync.dma_start(out=beta_pp, in_=beta.rearrange("(b p) -> p b", p=P))
        egrp_sb = consts.tile([P, 2 * G], F32, tag="egrp")
        nc.sync.dma_start(out=egrp_sb, in_=egrp)
        egrpt_sb = consts.tile([G, C], F32, tag="egrpt")
        nc.sync.dma_start(out=egrpt_sb, in_=egrpt)

        # ---- phase A: load x, stats, transpose ----
        xnT = big.tile([P, 2, N], F32, tag="xnT")

        with tc.tile_pool(name="xnat", bufs=1) as xnat_pool, \
             tc.tile_pool(name="psA", bufs=1, space="PSUM") as psA, \
             tc.tile_pool(name="psT", bufs=2, space="PSUM") as psT:
            x_nat = xnat_pool.tile([P, N // P, C], F32, tag="xnat")
            nc.sync.dma_start(out=x_nat, in_=x)

            stage = _STAGES[VARIANT]
            if stage < 1:
                return

            # raw transpose x -> xT (normalize applied in-place later)
            for t in range(N // P):
                for cb in range(2):
                    ps_t = psT.tile([P, P], F32, tag="tp")
                    nc.tensor.transpose(ps_t, x_nat[:, t, cb * P:(cb + 1) * P],
                                        ident_sb)
                    nc.vector.tensor_copy(out=xnT[:, cb, t * P:(t + 1) * P],
                                          in_=ps_t)

            if stage < 2:
                return

            # per-channel mean/var via bn_stats on x^T (channels on partitions)
            fmax = nc.vector.BN_STATS_FMAX
            nsub = N // fmax
            ps_g = psA.tile([G, 2], F32, tag="gstats")
            for cb in range(2):
                stats = work.tile([P, nsub, nc.vector.BN_STATS_DIM], F32,
                                  tag="bnstats")
                for s in range(nsub):
                    nc.vector.bn_stats(
                        out=stats[:, s, :],
                        in_=xnT[:, cb, s * fmax:(s + 1) * fmax])
                mv = work.tile([P, nc.vector.BN_AGGR_DIM], F32, tag="bnmv")
                nc.vector.bn_aggr(out=mv, in_=stats)
                # pack (mean_c, E[x^2]_c = var_c + mean_c^2)
                pk = work.tile([P, 2], F32, tag="pk")
                nc.vector.tensor_copy(out=pk[:, 0:1], in_=mv[:, 0:1])
                msq = work.tile([P, 1], F32, tag="msq")
                nc.vector.tensor_mul(out=msq, in0=mv[:, 0:1], in1=mv[:, 0:1])
                nc.vector.tensor_add(out=pk[:, 1:2], in0=mv[:, 1:2], in1=msq)
                # group-sum across channel partitions (one-hot matmul)
                nc.tensor.matmul(ps_g, lhsT=egrp_sb[:, cb * G:(cb + 1) * G],
                                 rhs=pk, start=(cb == 0), stop=(cb == 1),
                                 skip_group_check=True)

            # finalize on G partitions: mean_g, rstd_g
            gsb = small.tile([G, 2], F32, tag="gsb")
            nc.vector.tensor_scalar_mul(gsb, ps_g, 1.0 / CG)
            gmean = gsb[:, 0:1]
            gex2 = gsb[:, 1:2]
            gmsq = small.tile([G, 1], F32, tag="gmsq")
            nc.vector.tensor_mul(out=gmsq, in0=gmean, in1=gmean)
            gvar = small.tile([G, 1], F32, tag="gvar")
            nc.vector.tensor_tensor(out=gvar, in0=gex2, in1=gmsq,
                                    op=mybir.AluOpType.subtract)
            eps_sb = small.tile([G, 1], F32, tag="eps")
            nc.vector.memset(eps_sb, EPS)
            gstd = small.tile([G, 1], F32, tag="gstd")
            nc.scalar.activation(out=gstd, in_=gvar, func=AF.Sqrt, bias=eps_sb)
            gpack = small.tile([G, 2], F32, tag="gpack")
            nc.vector.tensor_copy(out=gpack[:, 0:1], in_=gmean)
            nc.vector.reciprocal(out=gpack[:, 1:2], in_=gstd)

            # broadcast group stats back to channel partitions
            all4 = consts.tile([P, 4], F32, tag="all4")
            for cb in range(2):
                ps_bc = psA.tile([P, 2], F32, tag="bc")
                nc.tensor.matmul(ps_bc, lhsT=egrpt_sb[:, cb * P:(cb + 1) * P],
                                 rhs=gpack, start=True, stop=True)
                mr = small.tile([P, 2], F32, tag="mr")
                nc.vector.tensor_copy(out=mr, in_=ps_bc)
                # scale_c = rstd_g(c) * gamma_c ; shift_c = beta_c - mean*scale
                nc.vector.tensor_mul(out=all4[:, cb:cb + 1], in0=mr[:, 1:2],
                                     in1=gamma_pp[:, cb:cb + 1])
                ms = small.tile([P, 1], F32, tag="ms")
                nc.vector.tensor_mul(out=ms, in0=mr[:, 0:1],
                                     in1=all4[:, cb:cb + 1])
                nc.vector.tensor_tensor(out=all4[:, 2 + cb:3 + cb],
                                        in0=beta_pp[:, cb:cb + 1], in1=ms,
                                        op=mybir.AluOpType.subtract)

            # normalize xT in place: xn^T = x^T * scale_c + shift_c
            for cb in range(2):
                nc.vector.tensor_scalar(
                    out=xnT[:, cb, :], in0=xnT[:, cb, :],
                    scalar1=all4[:, cb:cb + 1], scalar2=all4[:, 2 + cb:3 + cb],
                    op0=mybir.AluOpType.mult, op1=mybir.AluOpType.add)

        if _STAGES[VARIANT] < 3:
            return

        # ---- phase B: q^T, k^T (weights stationary), v natural ----
        qT = big.tile([P, 2, NQ], F32, tag="qT")
        kT = big.tile([P, 2, N], F32, tag="kT")
        v_sb = big.tile([P, N // P, C], F32, tag="v")

        with tc.tile_pool(name="psB", bufs=2, space="PSUM") as psB:
            for cob in range(2):
                for nt in range(NQ // FD):
                    ps = psB.tile([P, FD], F32, tag="qk")
                    for kb in range(2):
                        nc.tensor.matmul(
                            ps, lhsT=w_sbs["q"][:, kb, cob * P:(cob + 1) * P],
                            rhs=xnT[:, kb, nt * FD:(nt + 1) * FD],
                            start=(kb == 0), stop=(kb == 1))
                    nc.vector.tensor_scalar_add(
                        qT[:, cob, nt * FD:(nt + 1) * FD], ps,
                        bq_pp[:, cob:cob + 1])
                for nt in range(N // FD):
                    ps = psB.tile([P, FD], F32, tag="qk")
                    for kb in range(2):
                        nc.tensor.matmul(
                            ps, lhsT=w_sbs["k"][:, kb, cob * P:(cob + 1) * P],
                            rhs=xnT[:, kb, nt * FD:(nt + 1) * FD],
                            start=(kb == 0), stop=(kb == 1))
                    nc.vector.tensor_scalar_add(
                        kT[:, cob, nt * FD:(nt + 1) * FD], ps,
                        bk_pp[:, cob:cob + 1])
            for rb in range(N // P):
                psv = psB.tile([P, C], F32, tag="vps")
                for kb in range(2):
                    nc.tensor.matmul(psv, lhsT=xnT[:, kb, rb * P:(rb + 1) * P],
                                     rhs=w_sbs["v"][:, kb, :],
                                     start=(kb == 0), stop=(kb == 1))
                nc.vector.tensor_add(out=v_sb[:, rb, :], in0=psv, in1=bv_bc)

        if _STAGES[VARIANT] < 4:
            return

        # ---- phase C: attention per query tile of FD rows ----
        with tc.tile_pool(name="psS", bufs=2, space="PSUM") as psS, \
             tc.tile_pool(name="psO", bufs=1, space="PSUM") as psO, \
             tc.tile_pool(name="psC", bufs=1, space="PSUM") as psC, \
             tc.tile_pool(name="psM", bufs=1, space="PSUM") as psM, \
             tc.tile_pool(name="epool", bufs=4) as epool, \
             tc.tile_pool(name="apool", bufs=2) as apool, \
             tc.tile_pool(name="fpool", bufs=2) as fpool, \
             tc.tile_pool(name="opool", bufs=3) as opool:
            n_qt = {"ab": 0, "abs": NQT, "ab1": 1, "full": NQT}[VARIANT]
            scores_only = VARIANT == "abs"
            for qt in range(n_qt):
                qs = qt * FD
                if not scores_only:
                    ps_o0 = psO.tile([P, FD], F32, tag="o0")
                    ps_o1 = psO.tile([P, FD], F32, tag="o1")
                    ps_c = psC.tile([1, FD], F32, tag="c")
                for kb in range(NKB):
                    ps_s = psS.tile([P, FD], F32, tag="s")
                    nc.tensor.matmul(ps_s, lhsT=kT[:, 0, kb * P:(kb + 1) * P],
                                     rhs=qT[:, 0, qs:qs + FD],
                                     start=True, stop=False)
                    nc.tensor.matmul(ps_s, lhsT=kT[:, 1, kb * P:(kb + 1) * P],
                                     rhs=qT[:, 1, qs:qs + FD],
                                     start=False, stop=True)
                    eT = epool.tile([P, FD], F32, tag="eT")
                    nc.scalar.activation(out=eT, in_=ps_s, func=AF.Exp,
                                         scale=SCALE)
                    if scores_only:
                        continue
                    last = kb == NKB - 1
                    nc.tensor.matmul(ps_o0, lhsT=v_sb[:, kb, 0:P], rhs=eT,
                                     start=(kb == 0), stop=last,
                                     skip_group_check=True)
                    nc.tensor.matmul(ps_o1, lhsT=v_sb[:, kb, P:C], rhs=eT,
                                     start=(kb == 0), stop=last,
                                     skip_group_check=True)
                    nc.tensor.matmul(ps_c, lhsT=ones_sb, rhs=eT,
                                     start=(kb == 0), stop=last,
                                     skip_group_check=True)
                if scores_only:
                    continue
                # normalize: attn^T = outT_unnorm * (1/colsum) broadcast
                rec = small.tile([1, FD], F32, tag="rec")
                nc.vector.reciprocal(out=rec, in_=ps_c)
                ps_r = psM.tile([P, FD], F32, tag="r")
                nc.tensor.matmul(ps_r, lhsT=ones_row, rhs=rec,
                                 start=True, stop=True)
                rec_bc = work.tile([P, FD], F32, tag="rec_bc")
                nc.vector.tensor_copy(out=rec_bc, in_=ps_r)
                aT = apool.tile([P, 2, FD], F32, tag="aT")
                nc.vector.tensor_mul(out=aT[:, 0, :], in0=ps_o0, in1=rec_bc)
                nc.vector.tensor_mul(out=aT[:, 1, :], in0=ps_o1, in1=rec_bc)
                # proj (Wp stationary) + bp + residual xn^T
                fos = []
                for cob in range(2):
                    ps_f = psM.tile([P, FD], F32, tag="f")
                    for kb2 in range(2):
                        nc.tensor.matmul(
                            ps_f, lhsT=w_sbs["p"][:, kb2, cob * P:(cob + 1) * P],
                            rhs=aT[:, kb2, :], start=(kb2 == 0), stop=(kb2 == 1))
                    fo = fpool.tile([P, FD], F32, tag=f"fo{cob}")
                    nc.vector.tensor_scalar_add(fo, ps_f, bp_pp[:, cob:cob + 1])
                    nc.vector.tensor_add(out=fo, in0=fo,
                                         in1=xnT[:, cob, qs:qs + FD])
                    fos.append(fo)
                # transpose back to natural [rows, C] and store
                for qb in range(FD // P):
                    onat = opool.tile([P, C], F32, tag="onat")
                    for cob in range(2):
                        ps_ot = psM.tile([P, P], F32, tag="ot")
                        nc.tensor.transpose(ps_ot,
                                            fos[cob][:, qb * P:(qb + 1) * P],
                                            ident_sb)
                        nc.vector.tensor_copy(out=onat[:, cob * P:(cob + 1) * P],
                                              in_=ps_ot)
                    r0 = qs + qb * P
                    nc.sync.dma_start(out=out[r0:r0 + P, :], in_=onat)


_NC_CACHE = None


def _get_nc():
    global _NC_CACHE
    if _NC_CACHE is None:
        _NC_CACHE = build_nc()
    return _NC_CACHE


def _egrp_const() -> np.ndarray:
    """[P, 2G] one-hot: egrp[p, cb*G+g] = 1 iff channel cb*P+p is in group g."""
    e = np.zeros((P, 2 * G), dtype=np.float32)
    for cb in range(2):
        for p in range(P):
            e[p, cb * G + (cb * P + p) // CG] = 1.0
    return e


def _egrpt_const() -> np.ndarray:
    """[G, C] one-hot transpose: egrpt[g, c] = 1 iff group(c) == g."""
    e = np.zeros((G, C), dtype=np.float32)
    for c in range(C):
        e[c // CG, c] = 1.0
    return e


def make_in_maps(inputs: dict) -> list[dict]:
    x = np.ascontiguousarray(np.asarray(inputs["x"], dtype=np.float32))
    x_flat = x.reshape(B, N, C)
    shared = np.concatenate([
        np.asarray(inputs["Wq"], np.float32).ravel(),
        np.asarray(inputs["Wk"], np.float32).ravel(),
        np.asarray(inputs["Wv"], np.float32).ravel(),
        np.asarray(inputs["Wp"], np.float32).ravel(),
        np.asarray(inputs["bq"], np.float32).ravel(),
        np.asarray(inputs["bk"], np.float32).ravel(),
        np.asarray(inputs["bv"], np.float32).ravel(),
        np.asarray(inputs["bp"], np.float32).ravel(),
        np.asarray(inputs["gamma"], np.float32).ravel(),
        np.asarray(inputs["beta"], np.float32).ravel(),
        np.eye(P, dtype=np.float32).ravel(),
        _egrp_const().ravel(),
        _egrpt_const().ravel(),
    ])
    in_maps = []
    for core in range(8):
        b, h = core // 2, core % 2
        if h == 0:
            xp = x_flat[b]
        else:
            xp = np.concatenate([x_flat[b, NQ:], x_flat[b, :NQ]], axis=0)
        # blob layout: x in [p t c] order (partition-major), then the
        # shared weights/constants — must match _SEGS/_OFF
        xp_ptc = np.ascontiguousarray(
            xp.reshape(N // P, P, C).transpose(1, 0, 2)).ravel()
        in_maps.append({"blob": np.concatenate([xp_ptc, shared])})
    return in_maps


def assemble(results: list[dict]) -> np.ndarray:
    y = np.empty((B, N, C), dtype=np.float32)
    for core in range(8):
        b, h = core // 2, core % 2
        y[b, h * NQ:(h + 1) * NQ] = results[core]["out"]
    return y.reshape(B, H, W, C)


def kernel(**inputs) -> np.ndarray:
    nc = _get_nc()
    res = run_bass_kernel_spmd(nc, make_in_maps(inputs), core_ids=list(range(8)))
    return assemble(res.results)


# revision 30
# speedup vs baseline: 116.5990x; 27.2937x over previous
"""Self-contained Trainium2 Bass kernel for nn_AttentionBlock (GroupNorm +
single-head attention + residual).

Reference computation (shapes hardcoded):
    x: [B=4, H=64, W=64, C=256] f32
    xn = GroupNorm(x, groups=8, eps=1e-3) * gamma + beta
    q/k/v = xn @ W{q,k,v} + b{q,k,v}           # per batch, N=H*W=4096 tokens
    attn = softmax(q @ k^T / sqrt(C))
    out  = xn + (attn @ v) @ Wp + bp

Sharding: 8 cores = (batch b, query-half h). Each core receives its batch's
full token sequence with rows rotated so its 2048 query rows come first,
computes groupnorm + K/V for all 4096 tokens and attention for its 2048
query rows. No collectives. Host reassembles the 8 [2048, 256] outputs.
"""

import numpy as np

import concourse.bass as bass
import concourse.tile as tile
from concourse import mybir
from concourse.bass_utils import run_bass_kernel_spmd
from concourse.tile import ScopedClock

# Problem shapes (hardcoded per contract)
B, H, W, C = 4, 64, 64, 256
N = H * W            # 4096 tokens per batch image
NQ = N // 2          # 2048 query rows per core
G = 8                # groupnorm groups
CG = C // G          # 32 channels per group
EPS = 1e-3
P = 128
FD = 512             # matmul moving-operand free dim (one PSUM bank of f32)
NKB = N // P         # 32 key blocks
NQT = NQ // FD       # 4 query tiles per core
SCALE = float(C) ** -0.5
F32 = mybir.dt.float32
AF = mybir.ActivationFunctionType

# dev knob for phase bisection:
# "load" (DMAs only) | "ldtp" (+transposes) | "ldst" (+stats/normalize)
# | "ab" (+qkv) | "abs" (+scores/exp) | "ab1" (1 q-tile) | "full"
VARIANT = "full"
_STAGES = {"load": 0, "ldtp": 1, "ldst": 2, "ab": 3, "abs": 4, "ab1": 5,
           "full": 5}
# dev knob: repeat the whole body R times inside one NEFF (throughput probe)
REPS = 1


def _drain_and_barrier_split(self, tick_clock, wait_clock):
    """Replacement for TileContext._drain_and_barrier.

    The walrus build in this container rejects sem waits on InstDrain (and
    >1 wait on a NOP), so carry the end-of-kernel waits on a chain of NOPs
    with one wait each, drain without sync, and use the sem-only (no-Drain)
    all-engine barrier around semaphore cleanup.
    """
    nc = self.nc
    carrier = nc.sync.nop(nofuse=True)
    wait_clock.add_sem_waits(
        carrier.ins, ScopedClock({None: tick_clock.global_clock})
    )
    si = carrier.ins.sync_info
    waits = list(si.on_wait) if si is not None and si.on_wait else []
    if len(waits) > 1:
        carrier.ins.sync_info = mybir.SyncInfo(
            on_wait=waits[:1], on_update=list(si.on_update or [])
        )
        for w in waits[1:]:
            extra = nc.sync.nop(nofuse=True)
            extra.ins.sync_info = mybir.SyncInfo(on_wait=[w], on_update=[])
    nc.sync.drain()
    nc.all_engine_barrier(sem_only=True)
    assert self.sems is not None
    popped = nc._tile_sem_poison_stack.pop()
    assert popped is self._sem_poison
    nc.clear_and_free_semaphores(list(self.sems.allocated().values()))
    nc.all_engine_barrier(sem_only=True)


tile.TileContext._drain_and_barrier = _drain_and_barrier_split

_wsplit_ctr = 0


def _split_multi_waits(nc: bass.Bass):
    """Walrus in this container supports at most one sync wait per
    instruction (and none on Drain). Hoist excess waits onto NoOps placed
    just before the instruction on the same engine — sequencers process
    instructions in order, so blocking on the NoOp is equivalent."""
    global _wsplit_ctr
    for f in nc.m.functions:
        for bb in f.blocks:
            new_insts = []
            for ins in bb.instructions:
                si = getattr(ins, "sync_info", None)
                waits = list(si.on_wait) if si is not None and si.on_wait else []
                limit = 0 if ins.opcode == "Drain" else 1
                if len(waits) > limit:
                    keep = waits[len(waits) - limit:] if limit else []
                    hoist = waits[: len(waits) - limit]
                    for w in hoist:
                        _wsplit_ctr += 1
                        nop = mybir.InstNoOp(
                            name=f"I-wsplit-{_wsplit_ctr}",
                            engine=ins.engine,
                            sync_info=mybir.SyncInfo(on_wait=[w], on_update=[]),
                        )
                        new_insts.append(nop)
                    ins.sync_info = mybir.SyncInfo(
                        on_wait=keep, on_update=list(si.on_update or [])
                    )
                new_insts.append(ins)
            bb.instructions[:] = new_insts


# ---- single-blob input packing (one input param + one output param:
# each extra parameter costs ~2 ms/execution in this PJRT path) ----
_SEGS = [
    ("x", N * C),
    ("wq", C * C), ("wk", C * C), ("wv", C * C), ("wp", C * C),
    ("bq", C), ("bk", C), ("bv", C), ("bp", C),
    ("gamma", C), ("beta", C),
    ("ident", P * P),
    ("egrp", P * 2 * G),
    ("egrpt", G * C),
]
_OFF = {}
_total = 0
for _nm, _sz in _SEGS:
    _OFF[_nm] = _total
    _total += _sz
BLOB_SIZE = _total


def build_nc(split_waits: bool = True) -> bass.Bass:
    nc = bass.Bass(enable_partition_id=False)
    blob = nc.declare_dram_parameter("blob", [BLOB_SIZE], F32, isOutput=False)[:]

    def seg(name, size):
        return blob[_OFF[name]:_OFF[name] + size]

    x = seg("x", N * C).rearrange("(p t c) -> p t c", p=P, c=C)
    wq = seg("wq", C * C).rearrange("(ci co) -> ci co", co=C)
    wk = seg("wk", C * C).rearrange("(ci co) -> ci co", co=C)
    wv = seg("wv", C * C).rearrange("(ci co) -> ci co", co=C)
    wp = seg("wp", C * C).rearrange("(ci co) -> ci co", co=C)
    bq = seg("bq", C)
    bk = seg("bk", C)
    bv = seg("bv", C)
    bp = seg("bp", C)
    gamma = seg("gamma", C)
    beta = seg("beta", C)
    ident = seg("ident", P * P).rearrange("(a b) -> a b", b=P)
    egrp = seg("egrp", P * 2 * G).rearrange("(a b) -> a b", b=2 * G)
    egrpt = seg("egrpt", G * C).rearrange("(a b) -> a b", b=C)
    out = nc.declare_dram_parameter("out", [NQ, C], F32, isOutput=True)[:]

    with tile.TileContext(nc) as tc:
        for _rep in range(REPS):
            _body(nc, tc, x, wq, wk, wv, wp, bq, bk, bv, bp, gamma, beta,
                  ident, egrp, egrpt, out)
    if split_waits:
        _split_multi_waits(nc)
    return nc


def _body(nc, tc, x, wq, wk, wv, wp, bq, bk, bv, bp, gamma, beta, ident,
          egrp, egrpt, out):
    from contextlib import ExitStack
    ctx = ExitStack()
    with ctx:
        consts = ctx.enter_context(tc.tile_pool(name="consts", bufs=1))
        big = ctx.enter_context(tc.tile_pool(name="big", bufs=1))
        work = ctx.enter_context(tc.tile_pool(name="work", bufs=3))
        small = ctx.enter_context(tc.tile_pool(name="small", bufs=2))

        # ---- constants ----
        ident_sb = consts.tile([P, P], F32, tag="ident")
        nc.sync.dma_start(out=ident_sb, in_=ident)
        ones_sb = consts.tile([P, 1], F32, tag="ones")
        nc.vector.memset(ones_sb, 1.0)
        ones_row = consts.tile([1, P], F32, tag="ones_row")
        nc.vector.memset(ones_row, 1.0)

        w_sbs = {}
        for name, wh in (("q", wq), ("k", wk), ("v", wv), ("p", wp)):
            t = consts.tile([P, 2, C], F32, tag=f"w{name}")
            nc.sync.dma_start(out=t, in_=wh.rearrange("(kb p) co -> p kb co", p=P))
            w_sbs[name] = t
        bq_pp = consts.tile([P, 2], F32, tag="bq")
        nc.sync.dma_start(out=bq_pp, in_=bq.rearrange("(b p) -> p b", p=P))
        bk_pp = consts.tile([P, 2], F32, tag="bk")
        nc.sync.dma_start(out=bk_pp, in_=bk.rearrange("(b p) -> p b", p=P))
        bp_pp = consts.tile([P, 2], F32, tag="bp")
        nc.sync.dma_start(out=bp_pp, in_=bp.rearrange("(b p) -> p b", p=P))
        bv_bc = consts.tile([P, C], F32, tag="bv")
        nc.gpsimd.dma_start(
            out=bv_bc,
            in_=bass.AP(tensor=bv.tensor, offset=bv.offset, ap=[[0, P], [1, C]]),
        )
        gamma_pp = consts.tile([P, 2], F32, tag="gamma")
        nc.sync.dma_start(out=gamma_pp, in_=gamma.rearrange("(b p) -> p b", p=P))
        beta_pp = consts.tile([P, 2], F32, tag="beta")
        nc.sync.dma_start(out=beta_pp, in_=beta.rearrange("(b p) -> p b", p=P))
        egrp_sb = consts.tile([P, 2 * G], F32, tag="egrp")
        nc.sync.dma_start(out=egrp_sb, in_=egrp)
        egrpt_sb = consts.tile([G, C], F32, tag="egrpt")
        nc.sync.dma_start(out=egrpt_sb, in_=egrpt)

        # ---- phase A: load x, stats, transpose ----
        xnT = big.tile([P, 2, N], F32, tag="xnT")

        with tc.tile_pool(name="xnat", bufs=1) as xnat_pool, \
             tc.tile_pool(name="psA", bufs=1, space="PSUM") as psA, \
             tc.tile_pool(name="psT", bufs=2, space="PSUM") as psT:
            x_nat = xnat_pool.tile([P, N // P, C], F32, tag="xnat")
            nc.sync.dma_start(out=x_nat, in_=x)

            stage = _STAGES[VARIANT]
            if stage < 1:
                return

            # raw transpose x -> xT (normalize applied in-place later)
            for t in range(N // P):
                for cb in range(2):
                    ps_t = psT.tile([P, P], F32, tag="tp")
                    nc.tensor.transpose(ps_t, x_nat[:, t, cb * P:(cb + 1) * P],
                                        ident_sb)
                    nc.vector.tensor_copy(out=xnT[:, cb, t * P:(t + 1) * P],
                                          in_=ps_t)

            if stage < 2:
                return

            # per-channel mean/var via bn_stats on x^T (channels on partitions)
            fmax = nc.vector.BN_STATS_FMAX
            nsub = N // fmax
            ps_g = psA.tile([G, 2], F32, tag="gstats")
            for cb in range(2):
                stats = work.tile([P, nsub, nc.vector.BN_STATS_DIM], F32,
                                  tag="bnstats")
                for s in range(nsub):
                    nc.vector.bn_stats(
                        out=stats[:, s, :],
                        in_=xnT[:, cb, s * fmax:(s + 1) * fmax])
                mv = work.tile([P, nc.vector.BN_AGGR_DIM], F32, tag="bnmv")
                nc.vector.bn_aggr(out=mv, in_=stats)
                # pack (mean_c, E[x^2]_c = var_c + mean_c^2)
                pk = work.tile([P, 2], F32, tag="pk")
                nc.vector.tensor_copy(out=pk[:, 0:1], in_=mv[:, 0:1])
                msq = work.tile([P, 1], F32, tag="msq")
                nc.vector.tensor_mul(out=msq, in0=mv[:, 0:1], in1=mv[:, 0:1])
                nc.vector.tensor_add(out=pk[:, 1:2], in0=mv[:, 1:2], in1=msq)
                # group-sum across channel partitions (one-hot matmul)
                nc.tensor.matmul(ps_g, lhsT=egrp_sb[:, cb * G:(cb + 1) * G],
                                 rhs=pk, start=(cb == 0), stop=(cb == 1),
                                 skip_group_check=True)

            # finalize on G partitions: mean_g, rstd_g
            gsb = small.tile([G, 2], F32, tag="gsb")
            nc.vector.tensor_scalar_mul(gsb, ps_g, 1.0 / CG)
            gmean = gsb[:, 0:1]
            gex2 = gsb[:, 1:2]
            gmsq = small.tile([G, 1], F32, tag="gmsq")
            nc.vector.tensor_mul(out=gmsq, in0=gmean, in1=gmean)
            gvar = small.tile([G, 1], F32, tag="gvar")
            nc.vector.tensor_tensor(out=gvar, in0=gex2, in1=gmsq,
                                    op=mybir.AluOpType.subtract)
            eps_sb = small.tile([G, 1], F32, tag="eps")
            nc.vector.memset(eps_sb, EPS)
            gstd = small.tile([G, 1], F32, tag="gstd")
            nc.scalar.activation(out=gstd, in_=gvar, func=AF.Sqrt, bias=eps_sb)
            gpack = small.tile([G, 2], F32, tag="gpack")
            nc.vector.tensor_copy(out=gpack[:, 0:1], in_=gmean)
            nc.vector.reciprocal(out=gpack[:, 1:2], in_=gstd)

            # broadcast group stats back to channel partitions
            all4 = consts.tile([P, 4], F32, tag="all4")
            for cb in range(2):
                ps_bc = psA.tile([P, 2], F32, tag="bc")
                nc.tensor.matmul(ps_bc, lhsT=egrpt_sb[:, cb * P:(cb + 1) * P],
                                 rhs=gpack, start=True, stop=True)
                mr = small.tile([P, 2], F32, tag="mr")
                nc.vector.tensor_copy(out=mr, in_=ps_bc)
                # scale_c = rstd_g(c) * gamma_c ; shift_c = beta_c - mean*scale
                nc.vector.tensor_mul(out=all4[:, cb:cb + 1], in0=mr[:, 1:2],
                                     in1=gamma_pp[:, cb:cb + 1])
                ms = small.tile([P, 1], F32, tag="ms")
                nc.vector.tensor_mul(out=ms, in0=mr[:, 0:1],
                                     in1=all4[:, cb:cb + 1])
                nc.vector.tensor_tensor(out=all4[:, 2 + cb:3 + cb],
                                        in0=beta_pp[:, cb:cb + 1], in1=ms,
                                        op=mybir.AluOpType.subtract)

            # normalize xT in place: xn^T = x^T * scale_c + shift_c
            for cb in range(2):
                nc.vector.tensor_scalar(
                    out=xnT[:, cb, :], in0=xnT[:, cb, :],
                    scalar1=all4[:, cb:cb + 1], scalar2=all4[:, 2 + cb:3 + cb],
                    op0=mybir.AluOpType.mult, op1=mybir.AluOpType.add)

        if _STAGES[VARIANT] < 3:
            return

        # ---- phase B: q^T, k^T (weights stationary), v natural ----
        qT = big.tile([P, 2, NQ], F32, tag="qT")
        kT = big.tile([P, 2, N], F32, tag="kT")
        v_sb = big.tile([P, N // P, C], F32, tag="v")

        with tc.tile_pool(name="psB", bufs=2, space="PSUM") as psB:
            for cob in range(2):
                for nt in range(NQ // FD):
                    ps = psB.tile([P, FD], F32, tag="qk")
                    for kb in range(2):
                        nc.tensor.matmul(
                            ps, lhsT=w_sbs["q"][:, kb, cob * P:(cob + 1) * P],
                            rhs=xnT[:, kb, nt * FD:(nt + 1) * FD],
                            start=(kb == 0), stop=(kb == 1))
                    nc.vector.tensor_scalar_add(
                        qT[:, cob, nt * FD:(nt + 1) * FD], ps,
                        bq_pp[:, cob:cob + 1])
                for nt in range(N // FD):
                    ps = psB.tile([P, FD], F32, tag="qk")
                    for kb in range(2):
                        nc.tensor.matmul(
                            ps, lhsT=w_sbs["k"][:, kb, cob * P:(cob + 1) * P],
                            rhs=xnT[:, kb, nt * FD:(nt + 1) * FD],
                            start=(kb == 0), stop=(kb == 1))
                    nc.vector.tensor_scalar_add(
                        kT[:, cob, nt * FD:(nt + 1) * FD], ps,
                        bk_pp[:, cob:cob + 1])
            for rb in range(N // P):
                psv = psB.tile([P, C], F32, tag="vps")
                for kb in range(2):
                    nc.tensor.matmul(psv, lhsT=xnT[:, kb, rb * P:(rb + 1) * P],
                                     rhs=w_sbs["v"][:, kb, :],
                                     start=(kb == 0), stop=(kb == 1))
                nc.vector.tensor_add(out=v_sb[:, rb, :], in0=psv, in1=bv_bc)

        if _STAGES[VARIANT] < 4:
            return

        # ---- phase C: attention per query tile of FD rows ----
        with tc.tile_pool(name="psS", bufs=2, space="PSUM") as psS, \
             tc.tile_pool(name="psO", bufs=1, space="PSUM") as psO, \
             tc.tile_pool(name="psC", bufs=1, space="PSUM") as psC, \
             tc.tile_pool(name="psM", bufs=1, space="PSUM") as psM, \
             tc.tile_pool(name="epool", bufs=4) as epool, \
             tc.tile_pool(name="apool", bufs=2) as apool, \
             tc.tile_pool(name="fpool", bufs=2) as fpool, \
             tc.tile_pool(name="opool", bufs=3) as opool:
            n_qt = {"ab": 0, "abs": NQT, "ab1": 1, "full": NQT}[VARIANT]
            scores_only = VARIANT == "abs"
            for qt in range(n_qt):
                qs = qt * FD
                if not scores_only:
                    ps_o0 = psO.tile([P, FD], F32, tag="o0")
                    ps_o1 = psO.tile([P, FD], F32, tag="o1")
                    ps_c = psC.tile([1, FD], F32, tag="c")
                for kb in range(NKB):
                    ps_s = psS.tile([P, FD], F32, tag="s")
                    nc.tensor.matmul(ps_s, lhsT=kT[:, 0, kb * P:(kb + 1) * P],
                                     rhs=qT[:, 0, qs:qs + FD],
                                     start=True, stop=False)
                    nc.tensor.matmul(ps_s, lhsT=kT[:, 1, kb * P:(kb + 1) * P],
                                     rhs=qT[:, 1, qs:qs + FD],
                                     start=False, stop=True)
                    eT = epool.tile([P, FD], F32, tag="eT")
                    nc.scalar.activation(out=eT, in_=ps_s, func=AF.Exp,
                                         scale=SCALE)
                    if scores_only:
                        continue
                    last = kb == NKB - 1
                    nc.tensor.matmul(ps_o0, lhsT=v_sb[:, kb, 0:P], rhs=eT,
                                     start=(kb == 0), stop=last,
                                     skip_group_check=True)
                    nc.tensor.matmul(ps_o1, lhsT=v_sb[:, kb, P:C], rhs=eT,
                                     start=(kb == 0), stop=last,
                                     skip_group_check=True)
                    nc.tensor.matmul(ps_c, lhsT=ones_sb, rhs=eT,
                                     start=(kb == 0), stop=last,
                                     skip_group_check=True)
                if scores_only:
                    continue
                # normalize: attn^T = outT_unnorm * (1/colsum) broadcast
                rec = small.tile([1, FD], F32, tag="rec")
                nc.vector.reciprocal(out=rec, in_=ps_c)
                ps_r = psM.tile([P, FD], F32, tag="r")
                nc.tensor.matmul(ps_r, lhsT=ones_row, rhs=rec,
                                 start=True, stop=True)
                rec_bc = work.tile([P, FD], F32, tag="rec_bc")
                nc.vector.tensor_copy(out=rec_bc, in_=ps_r)
                aT = apool.tile([P, 2, FD], F32, tag="aT")
                nc.vector.tensor_mul(out=aT[:, 0, :], in0=ps_o0, in1=rec_bc)
                nc.vector.tensor_mul(out=aT[:, 1, :], in0=ps_o1, in1=rec_bc)
                # proj (Wp stationary) + bp + residual xn^T
                fos = []
                for cob in range(2):
                    ps_f = psM.tile([P, FD], F32, tag="f")
                    for kb2 in range(2):
                        nc.tensor.matmul(
                            ps_f, lhsT=w_sbs["p"][:, kb2, cob * P:(cob + 1) * P],
                            rhs=aT[:, kb2, :], start=(kb2 == 0), stop=(kb2 == 1))
                    fo = fpool.tile([P, FD], F32, tag=f"fo{cob}")
                    nc.vector.tensor_scalar_add(fo, ps_f, bp_pp[:, cob:cob + 1])
                    nc.vector.tensor_add(out=fo, in0=fo,
                                         in1=xnT[:, cob, qs:qs + FD])
                    fos.append(fo)
                # transpose back to natural [rows, C] and store
                for qb in range(FD // P):
                    onat = opool.tile([P, C], F32, tag="onat")
                    for cob in range(2):
                        ps_ot = psM.tile([P, P], F32, tag="ot")
                        nc.tensor.transpose(ps_ot,
                                            fos[cob][:, qb * P:(qb + 1) * P],
                                            ident_sb)
                        nc.vector.tensor_copy(out=onat[:, cob * P:(cob + 1) * P],
                                              in_=ps_ot)
                    r0 = qs + qb * P
                    nc.sync.dma_start(out=out[r0:r0 + P, :], in_=onat)


_NC_CACHE = None


def _get_nc():
    global _NC_CACHE
    if _NC_CACHE is None:
        _NC_CACHE = build_nc()
    return _NC_CACHE


_FN_CACHE = None


def _get_fn():
    """Compile once; return (fn, out_shape). fn takes the concatenated blob
    [8*BLOB_SIZE] plus a zero output buffer and runs all 8 cores."""
    global _FN_CACHE
    if _FN_CACHE is None:
        import jax
        from jax.experimental.shard_map import shard_map
        from jax.sharding import Mesh, PartitionSpec
        from concourse.bass2jax import (
            _bass_exec_p,
            install_neuronx_cc_hook,
            partition_id_tensor,
        )

        install_neuronx_cc_hook()
        nc = _get_nc()
        partition_name = (
            nc.partition_id_tensor.name if nc.partition_id_tensor else None
        )
        in_names, out_names, out_avals = [], [], []
        for alloc in nc.m.functions[0].allocations:
            if not isinstance(alloc, mybir.MemoryLocationSet):
                continue
            name = alloc.memorylocations[0].name
            if alloc.kind == "ExternalInput":
                if name != partition_name:
                    in_names.append(name)
            elif alloc.kind == "ExternalOutput":
                out_names.append(name)
                out_avals.append(
                    jax.core.ShapedArray(tuple(alloc.tensor_shape),
                                         mybir.dt.np(alloc.dtype)))
        assert in_names == ["blob"] and out_names == ["out"]
        all_in = in_names + out_names + (
            [partition_name] if partition_name else [])

        def _jbody(*args):
            ops = list(args)
            if partition_name:
                ops.append(partition_id_tensor())
            return tuple(_bass_exec_p.bind(
                *ops, out_avals=tuple(out_avals), in_names=tuple(all_in),
                out_names=tuple(out_names), lowering_input_output_aliases=(),
                sim_require_finite=True, sim_require_nnan=True, nc=nc))

        mesh = Mesh(np.asarray(jax.devices()[:8]), ("core",))
        fn = jax.jit(
            shard_map(_jbody, mesh=mesh,
                      in_specs=(PartitionSpec("core"),) * 2,
                      out_specs=(PartitionSpec("core"),), check_rep=False),
            keep_unused=True)
        _FN_CACHE = fn
    return _FN_CACHE


def _egrp_const() -> np.ndarray:
    """[P, 2G] one-hot: egrp[p, cb*G+g] = 1 iff channel cb*P+p is in group g."""
    e = np.zeros((P, 2 * G), dtype=np.float32)
    for cb in range(2):
        for p in range(P):
            e[p, cb * G + (cb * P + p) // CG] = 1.0
    return e


def _egrpt_const() -> np.ndarray:
    """[G, C] one-hot transpose: egrpt[g, c] = 1 iff group(c) == g."""
    e = np.zeros((G, C), dtype=np.float32)
    for c in range(C):
        e[c // CG, c] = 1.0
    return e


def make_in_maps(inputs: dict) -> list[dict]:
    x = np.ascontiguousarray(np.asarray(inputs["x"], dtype=np.float32))
    x_flat = x.reshape(B, N, C)
    shared = np.concatenate([
        np.asarray(inputs["Wq"], np.float32).ravel(),
        np.asarray(inputs["Wk"], np.float32).ravel(),
        np.asarray(inputs["Wv"], np.float32).ravel(),
        np.asarray(inputs["Wp"], np.float32).ravel(),
        np.asarray(inputs["bq"], np.float32).ravel(),
        np.asarray(inputs["bk"], np.float32).ravel(),
        np.asarray(inputs["bv"], np.float32).ravel(),
        np.asarray(inputs["bp"], np.float32).ravel(),
        np.asarray(inputs["gamma"], np.float32).ravel(),
        np.asarray(inputs["beta"], np.float32).ravel(),
        np.eye(P, dtype=np.float32).ravel(),
        _egrp_const().ravel(),
        _egrpt_const().ravel(),
    ])
    in_maps = []
    for core in range(8):
        b, h = core // 2, core % 2
        if h == 0:
            xp = x_flat[b]
        else:
            xp = np.concatenate([x_flat[b, NQ:], x_flat[b, :NQ]], axis=0)
        # blob layout: x in [p t c] order (partition-major), then the
        # shared weights/constants — must match _SEGS/_OFF
        xp_ptc = np.ascontiguousarray(
            xp.reshape(N // P, P, C).transpose(1, 0, 2)).ravel()
        in_maps.append({"blob": np.concatenate([xp_ptc, shared])})
    return in_maps


def assemble(results: list[dict]) -> np.ndarray:
    y = np.empty((B, N, C), dtype=np.float32)
    for core in range(8):
        b, h = core // 2, core % 2
        y[b, h * NQ:(h + 1) * NQ] = results[core]["out"]
    return y.reshape(B, H, W, C)


def kernel(**inputs) -> np.ndarray:
    fn = _get_fn()
    in_maps = make_in_maps(inputs)
    blob = np.concatenate([m["blob"] for m in in_maps])
    zeros = np.zeros((8 * NQ, C), np.float32)
    (out,) = fn(blob, zeros)
    out = np.asarray(out).reshape(8, NQ, C)
    return assemble([{"out": out[c]} for c in range(8)])
